# revision 29
# baseline (speedup 1.0000x reference)
"""Causal self-attention (B=4, T=2048, C=1024, 16 heads) on 8 trn2 cores.

Sharding: core c -> (batch b = c//2, head-group g = c%2 of 8 heads).
Each core computes qkv projection for its heads, causal attention, and a
partial c_proj product; the host sums the two partials per batch
(Megatron row-parallel reduce done at gather time).

Kernel layout (per core):
  - host supplies x[b].T (d-major), w slices pre-transposed, all bf16
  - qkv matmuls produce qT/kT d-major [64*2, T] per head-pair and V
    T-major [T, 8 heads, 64(+1 ones col)] for the AV matmul
  - attention computes S.T tiles [k=128 part, q<=512 free] = K Q^T,
    softmax without max-subtraction (S is O(5) so exp is safe),
    causal mask applied by accumulating a -1e9 strict-lower matrix into
    PSUM via an identity matmul (exp then underflows to 0)
  - AV: out.T[65, q] += [V|1].T @ P.T accumulated over k tiles; row 64
    is the softmax denominator (ones column trick)
  - normalize via DVE fast reciprocal + gpsimd partition_broadcast + DVE mul
  - c_proj: y.T = w_projT.T @ attT, partial over this core's channels,
    interleaved per T block with the next block's qkv/attention
"""

import math

import numpy as np
import ml_dtypes

B, T, C = 4, 2048, 1024
H = 16
D = 64
P = 128
HL = H // 2          # heads per core
NPAIR = HL // 2      # head pairs per core
KSUB = C // P        # 8 contraction subtiles for qkv
TB = 512             # T block (attention q block, qkv column block)
BF16 = ml_dtypes.bfloat16

NEG = -1.0e9
SCALE = 1.0 / math.sqrt(D)

_CACHE: dict = {}


def emit_attention(tc, io):
    """Emit the per-core kernel. io maps tensor name -> bass AP.

    Shapes (T_ may be reduced for simulation):
      xT      [C, T_]   bf16   x[b].T
      w_qk    [C, 1024] bf16   columns: [q pair0 | k pair0 | q pair1 | ...]
      w_v     [C, 512]  bf16   v weights for the 8 local heads, head-major
      w_pj    [512, C]  bf16   w_proj[:, local channels].T
      tri2    [128,2,128] bf16 two copies of the keep-mask (0 where q<k)
      yT      [C, T_]   bf16   output partial, transposed
    """
    from contextlib import ExitStack

    import concourse.mybir as mybir

    nc = tc.nc
    f32 = mybir.dt.float32
    bf = mybir.dt.bfloat16
    EXP = mybir.ActivationFunctionType.Exp

    xT, w_qk, w_v, w_pj = io["xT"], io["w_qk"], io["w_v"], io["w_pj"]
    tri2, yT = io["tri2"], io["yT"]

    T_ = xT.shape[1]
    NTB = T_ // TB       # number of 512-wide T blocks (= q blocks)
    NKT = T_ // P        # number of 128-row k tiles

    xT_r = xT.rearrange("(ko p) t -> p ko t", p=P)      # [128, 8, T]
    wqk_r = w_qk.rearrange("(ko p) n -> p ko n", p=P)   # [128, 8, 1024]
    wv_r = w_v.rearrange("(ko p) n -> p ko n", p=P)     # [128, 8, 512]
    wpj_r = w_pj.rearrange("(ko p) n -> p ko n", p=P)   # [128, 4, 1024]
    yT_r = yT.rearrange("(yt p) t -> p yt t", p=P)      # [128, 8, T]

    marks = []

    def mark(name):
        marks.append((name, nc.next_id()))

    with ExitStack() as ctx:
        const = ctx.enter_context(tc.tile_pool(name="const", bufs=1))
        persist = ctx.enter_context(tc.tile_pool(name="persist", bufs=1))
        work = ctx.enter_context(tc.tile_pool(name="work", bufs=3))
        psum = ctx.enter_context(tc.tile_pool(name="psum", bufs=3, space="PSUM"))

        # ---- constants ----
        # wqk + xt block 0 stream in ks-chunks so the first qkv accumulation
        # chain starts as soon as chunk 0 lands (subtile deps), instead of
        # the PE idling for the whole 5MB initial load.
        mark("setup")
        wqk_sb = const.tile([P, KSUB, 2 * HL * D], bf, tag="wqk")
        xt0 = work.tile([P, KSUB, TB], bf, tag="xt", bufs=2, name="xt_0")
        # small first chunk so the first qkv group starts ASAP; the rest in
        # one big transfer (each dma_start costs ~0.7us of sync-queue time).
        for c0, c1 in ((0, 2), (2, KSUB)):
            nc.sync.dma_start(wqk_sb[:, c0:c1, :], wqk_r[:, c0:c1, :])
            nc.sync.dma_start(xt0[:, c0:c1, :], xT_r[:, c0:c1, 0:TB])
        wv_sb = const.tile([P, KSUB, HL * D], bf, tag="wv")
        nc.sync.dma_start(wv_sb, wv_r)
        tri_sb = const.tile([P, 2, P], bf, tag="tri")
        nc.sync.dma_start(tri_sb, tri2)
        ones_sb = const.tile([1, P], bf, tag="ones")
        nc.gpsimd.memset(ones_sb, 1.0)
        wpj_sb = const.tile([P, HL * D // P, C], bf, tag="wpj")
        nc.sync.dma_start(wpj_sb, wpj_r)

        # ---- persistent intermediates ----
        qT_sb = [persist.tile([P, T_], bf, tag=f"qT{p}", name=f"qT{p}")
                 for p in range(NPAIR)]
        kT_sb = [persist.tile([P, T_], bf, tag=f"kT{p}", name=f"kT{p}")
                 for p in range(NPAIR)]
        # V in T-major laid out [1 | 0*63 | v*64] per head so that the AV
        # output's denominator row lands on PSUM partition 0 (where
        # reciprocal_approx_fast can read it) and the v rows span PSUM
        # partitions 64..127 (a >32-partition DVE read must start at 0 or 64).
        # M=128 costs nothing: matmul time is driven by the free dim only.
        VA = 128
        v_aug = persist.tile([P, NKT, HL, VA], bf, tag="vaug")
        nc.gpsimd.memset(v_aug[:, :, :, 0:64], 0.0)
        nc.gpsimd.memset(v_aug[:, :, :, 0], 1.0)
        attT_sb = persist.tile([P, NPAIR, T_], bf, tag="attT")

        emit_filler_ref = [lambda n=1: None]

        def attn_block(p, qb):
            """Attention for head pair p, query block qb (q in [qb*512, qb*512+512)).

            Software-pipelined: each unit (one st psum tile = 2 k-tiles for
            one head) emits its STs + exp immediately, but defers its AV
            matmuls until 2 more units' STs are on the PE queue. The PE then
            always has independent STs (h0/h1 run concurrently on disjoint
            PE row tiles) to execute while ACT computes the exp, instead of
            stalling on the exp latency at every unit.
            """
            from collections import deque
            av = [psum.tile([P, TB], f32, tag="av", bufs=2, name=f"av_{p}_{qb}_{h}")
                  for h in range(2)]
            n_full = 4 * qb
            pend = deque()

            def flush(n=1):
                for _ in range(n):
                    if pend:
                        pend.popleft()()

            def full_unit(i, h):
                d0, d1 = 64 * h, 64 * h + 64
                hg = 2 * p + h
                st = psum.tile([P, 2 * TB], f32, tag="st", bufs=3, name=f"st_{p}_{qb}_{i}_{h}")
                mark("stfull")
                for j in range(2):
                    kt = i + j
                    nc.tensor.matmul(
                        st[:, j * TB:(j + 1) * TB],
                        lhsT=kT_sb[p][d0:d1, kt * P:(kt + 1) * P],
                        rhs=qT_sb[p][d0:d1, qb * TB:(qb + 1) * TB],
                        start=True, stop=True,
                    )
                pt = work.tile([P, 2 * TB], bf, tag="pt", name=f"pt_{p}_{qb}_{i}_{h}")
                mark("exp")
                nc.scalar.activation(pt, st, EXP, scale=SCALE)

                def av_go():
                    mark("av")
                    for j in range(2):
                        kt = i + j
                        nc.tensor.matmul(
                            av[h][0:VA, :],
                            lhsT=v_aug[:, kt, hg, :],
                            rhs=pt[:, j * TB:(j + 1) * TB],
                            start=(kt == 0), stop=False,
                            skip_group_check=True,
                        )
                pend.append(av_go)

            def diag_unit(jp, h):
                # diagonal k tiles. Causal masking: the j-th diag tile only
                # computes q columns >= j*128, and the 128x128 triangle at
                # the left edge of each tile is zeroed AFTER exp by a single
                # strided DVE multiply with a 0/1 triangle (2 blocks/op).
                # j0/j1 share one psum accumulation group per 2KB zero
                # region when packed into the same bank (start=True zeroes
                # the whole region); jp=0 packs at 0/512 (two banks, two
                # groups), jp=1 at 0/256 (one bank, one start..stop group).
                j0 = 2 * jp
                w0 = TB - j0 * P            # width of j0 (512 or 256)
                w1 = w0 - P                 # width of j1 (384 or 128)
                d0, d1 = 64 * h, 64 * h + 64
                hg = 2 * p + h
                std = psum.tile([P, 2 * TB], f32, tag="st", bufs=3, name=f"std_{p}_{qb}_{jp}_{h}")
                same_bank = (w0 + w1) * 4 <= 2048
                mark("stdiag")
                for jj, (off, wdt) in enumerate(((0, w0), (w0, w1))):
                    j = j0 + jj
                    kt = n_full + j
                    nc.tensor.matmul(
                        std[:, off:off + wdt],
                        lhsT=kT_sb[p][d0:d1, kt * P:(kt + 1) * P],
                        rhs=qT_sb[p][d0:d1, qb * TB + j * P:(qb + 1) * TB],
                        start=(jj == 0 or not same_bank),
                        stop=(jj == 1 or not same_bank),
                        skip_group_check=True,
                    )
                ptd = work.tile([P, 2 * TB], bf, tag="pt", name=f"ptd_{p}_{qb}_{jp}_{h}")
                mark("expd")
                nc.scalar.activation(ptd[:, 0:w0 + w1], std[:, 0:w0 + w1], EXP, scale=SCALE)
                mark("trimask")
                blk = ptd[:, 0:2 * w0].rearrange("p (b c) -> p b c", b=2)[:, :, 0:P]
                nc.vector.tensor_mul(out=blk, in0=blk, in1=tri_sb)

                def av_go():
                    mark("avd")
                    for jj, (off, wdt) in enumerate(((0, w0), (w0, w1))):
                        j = j0 + jj
                        kt = n_full + j
                        nc.tensor.matmul(
                            av[h][0:VA, j * P:TB],
                            lhsT=v_aug[:, kt, hg, :],
                            rhs=ptd[:, off:off + wdt],
                            start=(kt == 0), stop=(j == 3),
                            skip_group_check=True,
                        )
                pend.append(av_go)

            # Fillers go at the block boundary: while the pipeline refills
            # (first 2 units' STs can issue, the 3rd waits for the previous
            # block's last exp to free an st slot) the PE has ~1.2us of
            # slack. Mid-block, ACT's exp rate paces the pipeline and the
            # PE is already balanced, so no fillers are needed there.
            emit_filler_ref[0](2)
            units = [("f", i, h) for i in range(0, n_full, 2) for h in (0, 1)]
            units += [("d", jp, h) for jp in (0, 1) for h in (0, 1)]
            for u in units:
                (full_unit if u[0] == "f" else diag_unit)(u[1], u[2])
                if len(pend) >= 3:
                    flush(1)
            while pend:
                flush(1)
                emit_filler_ref[0](1)

            # normalize by the denominator (AV row 0, on PSUM partition 0
            # where reciprocal_approx_fast can read it directly; it breaks at
            # base_part != 0) and store to attT. Reads the AV psum tile in
            # place — no sbuf staging copy.
            for h in range(2):
                mark("norm")
                # stash the unnormalized AV rows into attT right away (frees
                # the psum slot after one small cast + the 1-row reciprocal),
                # then scale attT in place by the broadcast reciprocal.
                dst = attT_sb[64 * h:64 * h + 64, p, qb * TB:(qb + 1) * TB]
                nc.vector.tensor_copy(out=dst, in_=av[h][64:64 + D, :])
                if qb == NTB - 1:
                    # last block: the final c_proj groups sit right behind
                    # this chain, so broadcast with a K=1 PE matmul
                    # (~0.2us) instead of the ~1us gpsimd broadcast.
                    rcp = work.tile([1, TB], f32, tag="rcp", bufs=2, name=f"rcp_{p}_{qb}_{h}")
                    nc.vector.reciprocal_approx_fast(out=rcp, in_=av[h][0:1, :])
                    rcpb = work.tile([1, TB], bf, tag="rcpb", bufs=2, name=f"rcpb_{p}_{qb}_{h}")
                    nc.vector.tensor_copy(out=rcpb, in_=rcp)
                    bcp = psum.tile([P, TB], f32, tag="av", bufs=2, name=f"bcp_{p}_{qb}_{h}")
                    nc.tensor.matmul(
                        bcp,
                        lhsT=ones_sb,
                        rhs=rcpb,
                        start=True, stop=True,
                    )
                    nc.vector.tensor_mul(
                        out=dst,
                        in0=dst,
                        in1=bcp[64 * h:64 * h + D, :],
                    )
                else:
                    rcp = work.tile([1, TB], f32, tag="rcp", bufs=2, name=f"rcp_{p}_{qb}_{h}")
                    nc.vector.reciprocal_approx_fast(out=rcp, in_=av[h][0:1, :])
                    bc = work.tile([P, TB], f32, tag="bc", bufs=2, name=f"bc_{p}_{qb}_{h}")
                    nc.gpsimd.partition_broadcast(bc, rcp)
                    nc.vector.tensor_mul(
                        out=dst,
                        in0=dst,
                        in1=bc[64 * h:64 * h + D, :],
                    )

        # ---- filler work: qkv / proj psum groups fed into attention stalls ----
        # The PE stream is in-order, so exp-wait bubbles inside the attention
        # stretch can only be filled by emitting independent matmul groups
        # between attention units. qkv of the NEXT T block and c_proj of the
        # PREVIOUS T block are both dependency-free at that point.
        from collections import deque
        filler_q = deque()

        def emit_filler(n=1):
            for _ in range(n):
                if filler_q:
                    filler_q.popleft()()

        emit_filler_ref[0] = emit_filler

        def qkv_qk_group(xt, tb, wt):
            def go():
                ps = psum.tile([P, TB], f32, tag="st", bufs=3, name=f"qk_ps_{tb}_{wt}")
                mark("qkvqk")
                for ks in range(KSUB):
                    nc.tensor.matmul(
                        ps,
                        lhsT=wqk_sb[:, ks, wt * P:(wt + 1) * P],
                        rhs=xt[:, ks, :],
                        start=(ks == 0), stop=(ks == KSUB - 1),
                    )
                pr, isk = divmod(wt, 2)
                dst = (kT_sb if isk else qT_sb)[pr][:, tb * TB:(tb + 1) * TB]
                mark("qkcopy")
                nc.vector.tensor_copy(out=dst, in_=ps)
            return go

        def qkv_v_group(xt, tb, tt):
            def go():
                psv = psum.tile([P, HL * D], f32, tag="st", bufs=3, name=f"v_ps_{tb}_{tt}")
                mark("qkvv")
                for ks in range(KSUB):
                    nc.tensor.matmul(
                        psv,
                        lhsT=xt[:, ks, tt * P:(tt + 1) * P],
                        rhs=wv_sb[:, ks, :],
                        start=(ks == 0), stop=(ks == KSUB - 1),
                    )
                kt_idx = tb * (TB // P) + tt
                mark("vcopy")
                nc.vector.tensor_copy(
                    out=v_aug[:, kt_idx, :, 64:64 + D],
                    in_=psv.rearrange("p (h d) -> p h d", h=HL),
                )
            return go

        def proj_group(tb, yrt):
            def go():
                pj = psum.tile([P, TB], f32, tag="st", bufs=3, name=f"pj_{yrt}_{tb}")
                mark("proj")
                for ks in range(NPAIR):
                    nc.tensor.matmul(
                        pj,
                        lhsT=wpj_sb[:, ks, yrt * P:(yrt + 1) * P],
                        rhs=attT_sb[:, ks, tb * TB:(tb + 1) * TB],
                        start=(ks == 0), stop=(ks == NPAIR - 1),
                    )
                mark("yout")
                yo = work.tile([P, TB], bf, tag="yo", name=f"yo_{yrt}_{tb}")
                nc.vector.tensor_copy(out=yo, in_=pj)
                nc.sync.dma_start(yT_r[:, yrt, tb * TB:(tb + 1) * TB], yo)
            return go

        def emit_xt_dma(tb):
            mark("xtdma")
            xt = work.tile([P, KSUB, TB], bf, tag="xt", bufs=2, name=f"xt_{tb}")
            nc.sync.dma_start(xt, xT_r[:, :, tb * TB:(tb + 1) * TB])
            return xt

        # ---- main loop ----
        # qkv(0) runs up front (v groups early so attn(0)'s AVs don't wait
        # for the stretch end); afterwards qkv(tb+1) and proj(tb-1) are
        # emitted as fillers inside attention(tb), paced so the filler
        # supply lasts the whole attention stretch instead of draining in
        # the first blocks and starving the last ones.
        for g in [qkv_qk_group(xt0, 0, wt) for wt in range(4)] + \
                 [qkv_v_group(xt0, 0, tt) for tt in range(TB // P)] + \
                 [qkv_qk_group(xt0, 0, wt) for wt in range(4, 2 * NPAIR)]:
            g()

        pace = {"acc": 0.0, "frac": 1.0}

        def emit_paced(n=1):
            pace["acc"] += n * pace["frac"]
            k = int(pace["acc"])
            pace["acc"] -= k
            emit_filler(k)

        emit_filler_ref[0] = emit_paced

        for tb in range(NTB):
            if tb + 1 < NTB:
                xt = emit_xt_dma(tb + 1)
                for wt in range(2 * NPAIR):
                    filler_q.append(qkv_qk_group(xt, tb + 1, wt))
                for tt in range(TB // P):
                    filler_q.append(qkv_v_group(xt, tb + 1, tt))
            if tb - 1 >= 0:
                for yrt in range(C // P):
                    filler_q.append(proj_group(tb - 1, yrt))

            sites = NPAIR * 5
            pace["frac"] = min(1.0, len(filler_q) / sites)
            for p in range(NPAIR):
                attn_block(p, qb=tb)

            while filler_q:
                emit_filler(1)

        for yrt in range(C // P):
            proj_group(NTB - 1, yrt)()

        mark("end")

    return marks


def _build(T_=T):
    if T_ in _CACHE:
        return _CACHE[T_]
    import concourse.bacc as bacc
    import concourse.mybir as mybir
    import concourse.tile as tile

    nc = bacc.Bacc("TRN2", debug=False, num_devices=8)
    bf = mybir.dt.bfloat16
    f32 = mybir.dt.float32
    io = {
        "xT": nc.dram_tensor("xT", [C, T_], bf, kind="ExternalInput").ap(),
        "w_qk": nc.dram_tensor("w_qk", [C, 2 * HL * D], bf, kind="ExternalInput").ap(),
        "w_v": nc.dram_tensor("w_v", [C, HL * D], bf, kind="ExternalInput").ap(),
        "w_pj": nc.dram_tensor("w_pj", [HL * D, C], bf, kind="ExternalInput").ap(),
        "tri2": nc.dram_tensor("tri2", [P, 2, P], bf, kind="ExternalInput").ap(),
        "yT": nc.dram_tensor("yT", [C, T_], bf, kind="ExternalOutput").ap(),
    }
    with tile.TileContext(nc) as tc:
        marks = emit_attention(tc, io)
    try:
        import json
        with open("/tmp/phase_marks.json", "w") as f:
            json.dump(marks, f)
    except Exception:
        pass
    nc.compile()
    _CACHE[T_] = nc
    return nc


def make_core_inputs(x, w_attn, w_proj, core, T_=T):
    """Host-side sharding for one core: (batch, head-group) slice + relayout."""
    b, g = divmod(core, 2)
    gs = slice(g * HL * D, (g + 1) * HL * D)
    q, k, v = w_attn[0:C], w_attn[C:2 * C], w_attn[2 * C:3 * C]
    qg, kg, vg = q[gs], k[gs], v[gs]          # [512, C] each
    blocks = []
    for p in range(NPAIR):
        blocks.append(qg[p * P:(p + 1) * P])
        blocks.append(kg[p * P:(p + 1) * P])
    wqk = np.concatenate(blocks, axis=0).T    # [C, 1024]
    return {
        "xT": np.ascontiguousarray(x[b, :T_].T).astype(BF16),
        "w_qk": np.ascontiguousarray(wqk).astype(BF16),
        "w_v": np.ascontiguousarray(vg.T).astype(BF16),
        "w_pj": np.ascontiguousarray(w_proj[:, gs].T).astype(BF16),
        "tri2": np.broadcast_to(
            np.triu(np.ones((P, P), np.float32))[:, None, :], (P, 2, P)
        ).astype(BF16).copy(),
    }


def kernel(x, w_attn, w_proj):
    x = np.asarray(x, dtype=np.float32)
    w_attn = np.asarray(w_attn, dtype=np.float32)
    w_proj = np.asarray(w_proj, dtype=np.float32)

    from concourse.bass_utils import run_bass_kernel_spmd

    nc = _build()
    in_maps = [make_core_inputs(x, w_attn, w_proj, c) for c in range(8)]
    res = run_bass_kernel_spmd(nc, in_maps, core_ids=list(range(8)))

    y = np.empty((B, T, C), dtype=np.float32)
    for b in range(B):
        yT = (res.results[2 * b]["yT"].astype(np.float32)
              + res.results[2 * b + 1]["yT"].astype(np.float32))
        y[b] = yT.T
    return y



# revision 32
# speedup vs baseline: 1.0404x; 1.0404x over previous
"""Causal self-attention (B=4, T=2048, C=1024, 16 heads) on 8 trn2 cores.

Sharding: core c -> (batch b = c//2, head-group g = c%2 of 8 heads).
Each core computes qkv projection for its heads, causal attention, and a
partial c_proj product; the host sums the two partials per batch
(Megatron row-parallel reduce done at gather time).

Kernel layout (per core):
  - host supplies x[b].T (d-major), w slices pre-transposed, all bf16
  - qkv matmuls produce qT/kT d-major [64*2, T] per head-pair and V
    T-major [T, 8 heads, 64(+1 ones col)] for the AV matmul
  - attention computes S.T tiles [k=128 part, q<=512 free] = K Q^T,
    softmax without max-subtraction (S is O(5) so exp is safe),
    causal mask applied by accumulating a -1e9 strict-lower matrix into
    PSUM via an identity matmul (exp then underflows to 0)
  - AV: out.T[65, q] += [V|1].T @ P.T accumulated over k tiles; row 64
    is the softmax denominator (ones column trick)
  - normalize via DVE fast reciprocal + gpsimd partition_broadcast + DVE mul
  - c_proj: y.T = w_projT.T @ attT, partial over this core's channels,
    interleaved per T block with the next block's qkv/attention
"""

import math

import numpy as np
import ml_dtypes

B, T, C = 4, 2048, 1024
H = 16
D = 64
P = 128
HL = H // 2          # heads per core
NPAIR = HL // 2      # head pairs per core
KSUB = C // P        # 8 contraction subtiles for qkv
TB = 512             # T block (attention q block, qkv column block)
BF16 = ml_dtypes.bfloat16

NEG = -1.0e9
SCALE = 1.0 / math.sqrt(D)

_CACHE: dict = {}


def emit_attention(tc, io):
    """Emit the per-core kernel. io maps tensor name -> bass AP.

    Shapes (T_ may be reduced for simulation):
      xT      [C, T_]   bf16   x[b].T
      w_qk    [C, 1024] bf16   columns: [q pair0 | k pair0 | q pair1 | ...]
      w_v     [C, 512]  bf16   v weights for the 8 local heads, head-major
      w_pj    [512, C]  bf16   w_proj[:, local channels].T
      tri2    [128,2,128] bf16 two copies of the keep-mask (0 where q<k)
      yT      [C, T_]   bf16   output partial, transposed
    """
    from contextlib import ExitStack

    import concourse.mybir as mybir

    nc = tc.nc
    f32 = mybir.dt.float32
    bf = mybir.dt.bfloat16
    EXP = mybir.ActivationFunctionType.Exp

    xT, w_qk, w_v, w_pj = io["xT"], io["w_qk"], io["w_v"], io["w_pj"]
    tri2, yT = io["tri2"], io["yT"]

    T_ = xT.shape[1]
    NTB = T_ // TB       # number of 512-wide T blocks (= q blocks)
    NKT = T_ // P        # number of 128-row k tiles

    xT_r = xT.rearrange("(ko p) t -> p ko t", p=P)      # [128, 8, T]
    wqk_r = w_qk.rearrange("(ko p) n -> p ko n", p=P)   # [128, 8, 1024]
    wv_r = w_v.rearrange("(ko p) n -> p ko n", p=P)     # [128, 8, 512]
    wpj_r = w_pj.rearrange("(ko p) n -> p ko n", p=P)   # [128, 4, 1024]
    yT_r = yT.rearrange("(yt p) t -> p yt t", p=P)      # [128, 8, T]

    marks = []

    def mark(name):
        marks.append((name, nc.next_id()))

    with ExitStack() as ctx:
        const = ctx.enter_context(tc.tile_pool(name="const", bufs=1))
        persist = ctx.enter_context(tc.tile_pool(name="persist", bufs=1))
        work = ctx.enter_context(tc.tile_pool(name="work", bufs=3))
        psum = ctx.enter_context(tc.tile_pool(name="psum", bufs=3, space="PSUM"))

        # ---- constants ----
        # wqk + xt block 0 stream in ks-chunks so the first qkv accumulation
        # chain starts as soon as chunk 0 lands (subtile deps), instead of
        # the PE idling for the whole 5MB initial load.
        mark("setup")
        wqk_sb = const.tile([P, KSUB, 2 * HL * D], bf, tag="wqk")
        xt0 = work.tile([P, KSUB, TB], bf, tag="xt", bufs=2, name="xt_0")
        hk = KSUB // 2
        for c in range(0, KSUB, hk):
            nc.sync.dma_start(wqk_sb[:, c:c + hk, :], wqk_r[:, c:c + hk, :])
            nc.sync.dma_start(xt0[:, c:c + hk, :], xT_r[:, c:c + hk, 0:TB])
        wv_sb = const.tile([P, KSUB, HL * D], bf, tag="wv")
        nc.sync.dma_start(wv_sb, wv_r)
        tri_sb = const.tile([P, 2, P], bf, tag="tri")
        nc.sync.dma_start(tri_sb, tri2)
        ones_sb = const.tile([1, P], bf, tag="ones")
        nc.gpsimd.memset(ones_sb, 1.0)
        wpj_sb = const.tile([P, HL * D // P, C], bf, tag="wpj")
        nc.sync.dma_start(wpj_sb, wpj_r)

        # ---- persistent intermediates ----
        qT_sb = [persist.tile([P, T_], bf, tag=f"qT{p}", name=f"qT{p}")
                 for p in range(NPAIR)]
        kT_sb = [persist.tile([P, T_], bf, tag=f"kT{p}", name=f"kT{p}")
                 for p in range(NPAIR)]
        # V in T-major laid out [1 | 0*63 | v*64] per head so that the AV
        # output's denominator row lands on PSUM partition 0 (where
        # reciprocal_approx_fast can read it) and the v rows span PSUM
        # partitions 64..127 (a >32-partition DVE read must start at 0 or 64).
        # M=128 costs nothing: matmul time is driven by the free dim only.
        VA = 128
        v_aug = persist.tile([P, NKT, HL, VA], bf, tag="vaug")
        nc.gpsimd.memset(v_aug[:, :, :, 0:64], 0.0)
        nc.gpsimd.memset(v_aug[:, :, :, 0], 1.0)
        attT_sb = persist.tile([P, NPAIR, T_], bf, tag="attT")

        emit_filler_ref = [lambda n=1: None]

        def attn_block(p, qb):
            """Attention for head pair p, query block qb (q in [qb*512, qb*512+512))."""
            av = [psum.tile([P, TB], f32, tag="av", bufs=2, name=f"av_{p}_{qb}_{h}")
                  for h in range(2)]
            n_full = 4 * qb

            # full k tiles, processed in pairs sharing one 2-bank psum tile.
            # Per-head chains (ST -> exp -> AV); h0/h1 STs run concurrently
            # on disjoint PE row tiles, and walrus hoists the interleaved
            # filler matmuls into the exp-latency bubbles.
            for i in range(0, n_full, 2):
                for h in range(2):
                    d0, d1 = 64 * h, 64 * h + 64
                    hg = 2 * p + h
                    st = psum.tile([P, 2 * TB], f32, tag="st", bufs=3, name=f"st_{p}_{qb}_{i}_{h}")
                    mark("stfull")
                    for j in range(2):
                        kt = i + j
                        nc.tensor.matmul(
                            st[:, j * TB:(j + 1) * TB],
                            lhsT=kT_sb[p][d0:d1, kt * P:(kt + 1) * P],
                            rhs=qT_sb[p][d0:d1, qb * TB:(qb + 1) * TB],
                            start=True, stop=True,
                        )
                    pt = work.tile([P, 2 * TB], bf, tag="pt", name=f"pt_{p}_{qb}_{i}_{h}")
                    mark("exp")
                    nc.scalar.activation(pt, st, EXP, scale=SCALE)
                    mark("av")
                    for j in range(2):
                        kt = i + j
                        nc.tensor.matmul(
                            av[h][0:VA, :],
                            lhsT=v_aug[:, kt, hg, :],
                            rhs=pt[:, j * TB:(j + 1) * TB],
                            start=(kt == 0), stop=False,
                            skip_group_check=True,
                        )
                    emit_filler_ref[0](1)

            # diagonal k tiles. Causal masking: the j-th diag tile only
            # computes q columns >= j*128, and the 128x128 triangle at the
            # left edge of each tile is zeroed AFTER exp by a single strided
            # DVE multiply with a 0/1 triangle (2 blocks per op). j0/j1
            # share one psum accumulation group per 2KB zero region when
            # packed into the same bank (start=True zeroes the whole
            # region); jp=0 packs at 0/512 (two banks, two groups), jp=1 at
            # 0/256 (one bank, one start..stop group).
            for jp in range(2):
                j0 = 2 * jp
                w0 = TB - j0 * P            # width of j0 (512 or 256)
                w1 = w0 - P                 # width of j1 (384 or 128)
                for h in range(2):
                    d0, d1 = 64 * h, 64 * h + 64
                    hg = 2 * p + h
                    std = psum.tile([P, 2 * TB], f32, tag="st", bufs=3, name=f"std_{p}_{qb}_{jp}_{h}")
                    same_bank = (w0 + w1) * 4 <= 2048
                    mark("stdiag")
                    for jj, (off, wdt) in enumerate(((0, w0), (w0, w1))):
                        j = j0 + jj
                        kt = n_full + j
                        nc.tensor.matmul(
                            std[:, off:off + wdt],
                            lhsT=kT_sb[p][d0:d1, kt * P:(kt + 1) * P],
                            rhs=qT_sb[p][d0:d1, qb * TB + j * P:(qb + 1) * TB],
                            start=(jj == 0 or not same_bank),
                            stop=(jj == 1 or not same_bank),
                            skip_group_check=True,
                        )
                    ptd = work.tile([P, 2 * TB], bf, tag="pt", name=f"ptd_{p}_{qb}_{jp}_{h}")
                    mark("expd")
                    nc.scalar.activation(ptd[:, 0:w0 + w1], std[:, 0:w0 + w1], EXP, scale=SCALE)
                    mark("trimask")
                    blk = ptd[:, 0:2 * w0].rearrange("p (b c) -> p b c", b=2)[:, :, 0:P]
                    nc.vector.tensor_mul(out=blk, in0=blk, in1=tri_sb)
                    mark("avd")
                    for jj, (off, wdt) in enumerate(((0, w0), (w0, w1))):
                        j = j0 + jj
                        kt = n_full + j
                        nc.tensor.matmul(
                            av[h][0:VA, j * P:TB],
                            lhsT=v_aug[:, kt, hg, :],
                            rhs=ptd[:, off:off + wdt],
                            start=(kt == 0), stop=(j == 3),
                            skip_group_check=True,
                        )
                    if h == 1:
                        emit_filler_ref[0](1)

            # normalize by the denominator (AV row 0, on PSUM partition 0
            # where reciprocal_approx_fast can read it directly; it breaks at
            # base_part != 0) and store to attT. Reads the AV psum tile in
            # place — no sbuf staging copy.
            for h in range(2):
                mark("norm")
                # stash the unnormalized AV rows into attT right away (frees
                # the psum slot after one small cast + the 1-row reciprocal),
                # then scale attT in place by the broadcast reciprocal.
                dst = attT_sb[64 * h:64 * h + 64, p, qb * TB:(qb + 1) * TB]
                nc.vector.tensor_copy(out=dst, in_=av[h][64:64 + D, :])
                if qb == NTB - 1:
                    # last block: the final c_proj groups sit right behind
                    # this chain, so broadcast with a K=1 PE matmul
                    # (~0.2us) instead of the ~1us gpsimd broadcast.
                    rcp = work.tile([1, TB], f32, tag="rcp", bufs=2, name=f"rcp_{p}_{qb}_{h}")
                    nc.vector.reciprocal_approx_fast(out=rcp, in_=av[h][0:1, :])
                    rcpb = work.tile([1, TB], bf, tag="rcpb", bufs=2, name=f"rcpb_{p}_{qb}_{h}")
                    nc.vector.tensor_copy(out=rcpb, in_=rcp)
                    bcp = psum.tile([P, TB], f32, tag="av", bufs=2, name=f"bcp_{p}_{qb}_{h}")
                    nc.tensor.matmul(
                        bcp,
                        lhsT=ones_sb,
                        rhs=rcpb,
                        start=True, stop=True,
                    )
                    nc.vector.tensor_mul(
                        out=dst,
                        in0=dst,
                        in1=bcp[64 * h:64 * h + D, :],
                    )
                else:
                    rcp = work.tile([1, TB], f32, tag="rcp", bufs=2, name=f"rcp_{p}_{qb}_{h}")
                    nc.vector.reciprocal_approx_fast(out=rcp, in_=av[h][0:1, :])
                    bc = work.tile([P, TB], f32, tag="bc", bufs=2, name=f"bc_{p}_{qb}_{h}")
                    nc.gpsimd.partition_broadcast(bc, rcp)
                    nc.vector.tensor_mul(
                        out=dst,
                        in0=dst,
                        in1=bc[64 * h:64 * h + D, :],
                    )

        # ---- filler work: qkv / proj psum groups fed into attention stalls ----
        # The PE stream is in-order, so exp-wait bubbles inside the attention
        # stretch can only be filled by emitting independent matmul groups
        # between attention units. qkv of the NEXT T block and c_proj of the
        # PREVIOUS T block are both dependency-free at that point.
        from collections import deque
        filler_q = deque()

        def emit_filler(n=1):
            for _ in range(n):
                if filler_q:
                    filler_q.popleft()()

        emit_filler_ref[0] = emit_filler

        def qkv_qk_group(xt, tb, wt):
            def go():
                ps = psum.tile([P, TB], f32, tag="st", bufs=3, name=f"qk_ps_{tb}_{wt}")
                mark("qkvqk")
                for ks in range(KSUB):
                    nc.tensor.matmul(
                        ps,
                        lhsT=wqk_sb[:, ks, wt * P:(wt + 1) * P],
                        rhs=xt[:, ks, :],
                        start=(ks == 0), stop=(ks == KSUB - 1),
                    )
                pr, isk = divmod(wt, 2)
                dst = (kT_sb if isk else qT_sb)[pr][:, tb * TB:(tb + 1) * TB]
                mark("qkcopy")
                nc.vector.tensor_copy(out=dst, in_=ps)
            return go

        def qkv_v_group(xt, tb, tt):
            def go():
                psv = psum.tile([P, HL * D], f32, tag="st", bufs=3, name=f"v_ps_{tb}_{tt}")
                mark("qkvv")
                for ks in range(KSUB):
                    nc.tensor.matmul(
                        psv,
                        lhsT=xt[:, ks, tt * P:(tt + 1) * P],
                        rhs=wv_sb[:, ks, :],
                        start=(ks == 0), stop=(ks == KSUB - 1),
                    )
                kt_idx = tb * (TB // P) + tt
                mark("vcopy")
                nc.vector.tensor_copy(
                    out=v_aug[:, kt_idx, :, 64:64 + D],
                    in_=psv.rearrange("p (h d) -> p h d", h=HL),
                )
            return go

        def proj_group(tb, yrt):
            def go():
                pj = psum.tile([P, TB], f32, tag="st", bufs=3, name=f"pj_{yrt}_{tb}")
                mark("proj")
                for ks in range(NPAIR):
                    nc.tensor.matmul(
                        pj,
                        lhsT=wpj_sb[:, ks, yrt * P:(yrt + 1) * P],
                        rhs=attT_sb[:, ks, tb * TB:(tb + 1) * TB],
                        start=(ks == 0), stop=(ks == NPAIR - 1),
                    )
                mark("yout")
                yo = work.tile([P, TB], bf, tag="yo", name=f"yo_{yrt}_{tb}")
                nc.vector.tensor_copy(out=yo, in_=pj)
                nc.sync.dma_start(yT_r[:, yrt, tb * TB:(tb + 1) * TB], yo)
            return go

        def emit_xt_dma(tb):
            mark("xtdma")
            xt = work.tile([P, KSUB, TB], bf, tag="xt", bufs=2, name=f"xt_{tb}")
            nc.sync.dma_start(xt, xT_r[:, :, tb * TB:(tb + 1) * TB])
            return xt

        # ---- main loop ----
        # qkv(0) runs up front (v groups early so attn(0)'s AVs don't wait
        # for the stretch end); afterwards qkv(tb+1) and proj(tb-1) are
        # emitted as fillers inside attention(tb), paced so the filler
        # supply lasts the whole attention stretch instead of draining in
        # the first blocks and starving the last ones.
        for g in [qkv_qk_group(xt0, 0, wt) for wt in range(4)] + \
                 [qkv_v_group(xt0, 0, tt) for tt in range(TB // P)] + \
                 [qkv_qk_group(xt0, 0, wt) for wt in range(4, 2 * NPAIR)]:
            g()

        emit_filler_ref[0] = emit_filler

        for tb in range(NTB):
            if tb + 1 < NTB:
                xt = emit_xt_dma(tb + 1)
                for wt in range(2 * NPAIR):
                    filler_q.append(qkv_qk_group(xt, tb + 1, wt))
                for tt in range(TB // P):
                    filler_q.append(qkv_v_group(xt, tb + 1, tt))
            if tb - 1 >= 0:
                for yrt in range(C // P):
                    filler_q.append(proj_group(tb - 1, yrt))

            for p in range(NPAIR):
                attn_block(p, qb=tb)
                emit_filler(2)

            while filler_q:
                emit_filler(1)

        for yrt in range(C // P):
            proj_group(NTB - 1, yrt)()

        mark("end")

    return marks


def _build(T_=T):
    if T_ in _CACHE:
        return _CACHE[T_]
    import concourse.bacc as bacc
    import concourse.mybir as mybir
    import concourse.tile as tile

    nc = bacc.Bacc("TRN2", debug=False, num_devices=8)
    bf = mybir.dt.bfloat16
    f32 = mybir.dt.float32
    io = {
        "xT": nc.dram_tensor("xT", [C, T_], bf, kind="ExternalInput").ap(),
        "w_qk": nc.dram_tensor("w_qk", [C, 2 * HL * D], bf, kind="ExternalInput").ap(),
        "w_v": nc.dram_tensor("w_v", [C, HL * D], bf, kind="ExternalInput").ap(),
        "w_pj": nc.dram_tensor("w_pj", [HL * D, C], bf, kind="ExternalInput").ap(),
        "tri2": nc.dram_tensor("tri2", [P, 2, P], bf, kind="ExternalInput").ap(),
        "yT": nc.dram_tensor("yT", [C, T_], bf, kind="ExternalOutput").ap(),
    }
    with tile.TileContext(nc) as tc:
        marks = emit_attention(tc, io)
    try:
        import json
        with open("/tmp/phase_marks.json", "w") as f:
            json.dump(marks, f)
    except Exception:
        pass
    nc.compile()
    _CACHE[T_] = nc
    return nc


def make_core_inputs(x, w_attn, w_proj, core, T_=T):
    """Host-side sharding for one core: (batch, head-group) slice + relayout."""
    b, g = divmod(core, 2)
    gs = slice(g * HL * D, (g + 1) * HL * D)
    q, k, v = w_attn[0:C], w_attn[C:2 * C], w_attn[2 * C:3 * C]
    qg, kg, vg = q[gs], k[gs], v[gs]          # [512, C] each
    blocks = []
    for p in range(NPAIR):
        blocks.append(qg[p * P:(p + 1) * P])
        blocks.append(kg[p * P:(p + 1) * P])
    wqk = np.concatenate(blocks, axis=0).T    # [C, 1024]
    return {
        "xT": np.ascontiguousarray(x[b, :T_].T).astype(BF16),
        "w_qk": np.ascontiguousarray(wqk).astype(BF16),
        "w_v": np.ascontiguousarray(vg.T).astype(BF16),
        "w_pj": np.ascontiguousarray(w_proj[:, gs].T).astype(BF16),
        "tri2": np.broadcast_to(
            np.triu(np.ones((P, P), np.float32))[:, None, :], (P, 2, P)
        ).astype(BF16).copy(),
    }


def kernel(x, w_attn, w_proj):
    x = np.asarray(x, dtype=np.float32)
    w_attn = np.asarray(w_attn, dtype=np.float32)
    w_proj = np.asarray(w_proj, dtype=np.float32)

    from concourse.bass_utils import run_bass_kernel_spmd

    nc = _build()
    in_maps = [make_core_inputs(x, w_attn, w_proj, c) for c in range(8)]
    res = run_bass_kernel_spmd(nc, in_maps, core_ids=list(range(8)))

    y = np.empty((B, T, C), dtype=np.float32)
    for b in range(B):
        yT = (res.results[2 * b]["yT"].astype(np.float32)
              + res.results[2 * b + 1]["yT"].astype(np.float32))
        y[b] = yT.T
    return y

